# revision 14
# baseline (speedup 1.0000x reference)
"""DetectionConfidenceMap2keypoint Trainium2 kernel (8 NeuronCores).

For x = combined_hm_preds [B=32, K=64, H=128, W=128] f32:
  map_val_all = softmax(x, axis=1)                 # over K, per (b,h,w)
  get_zeta[b,k]  = sum_{h,w} map
  kp_x[b,k]      = sum_{h,w} w * map
  kp_y[b,k]      = sum_{h,w} h * map
  keypoint[b,k]  = (round(kp_x/zeta), round(kp_y/zeta))

Sharding: data-parallel over batch, 4 batches per core on 8 cores (no
communication).

Device layout per batch: SBUF tile [h=128 partitions, (k,w)=8192 free].
Raw bass (no Tile) with a hand-scheduled pipeline and standalone
semaphore waits (this container's walrus encodes at most one inline
wait per instruction, which Tile's scheduler exceeds):
  SP  : dma-in  x[b] -> xt[b%2]
  ACT : exp xt->et; psum drain copies; map dma-out (ACT HWDGE ring)
  DVE : denom[h,w] = reduce_k(exp); recip; map = exp*recip (k-bcast)
  PE  : [ones | h]^T @ map -> colsum/ycolsum per (k,w)  (h-reduction)
  PL  : csums dma-out
Host: w-sums of colsums -> zeta/kp_x/kp_y; divide + round -> keypoint.
"""

import os
from contextlib import ExitStack

import numpy as np

import concourse.bass as bass
import concourse.mybir as mybir
from concourse.bass_utils import run_bass_kernel_spmd

B, K, H, W = 32, 64, 128, 128
MKD = 36  # k's normalized on DVE; rest on Pool (Pool ~2x slower/elem)
NCORES = 8
BLOC = B // NCORES
FREE = K * W  # 8192
HALF = FREE // 2  # 4096
KH = K // 2  # 32 k's per input half
NQ = 4
GQ_TOT = BLOC * NQ
NSQ = 6

_cache = {}



def _build():
    if "nc" in _cache:
        return _cache["nc"]

    nc = bass.Bass(
        target_bir_lowering=False,
        debug=False,
        num_devices=NCORES,
        dynamic_dma_scratch_size=8192,
    )

    x = nc.dram_tensor(
        "x", [BLOC, K, H, W], mybir.dt.float32, kind="ExternalInput"
    ).ap()
    map_out = nc.dram_tensor(
        "map_out", [BLOC, K, H, W], mybir.dt.float32, kind="ExternalOutput"
    ).ap()
    csums = nc.dram_tensor(
        "csums", [BLOC, 2, K, W], mybir.dt.float32, kind="ExternalOutput"
    ).ap()

    wgt_np = np.zeros((128, 2), dtype=np.float32)
    wgt_np[:, 0] = 1.0
    wgt_np[:, 1] = np.arange(128, dtype=np.float32)
    wgt_dram = nc.inline_tensor(wgt_np, "wgt").ap()

    ctx = ExitStack()
    with ctx:
        sb = lambda name, shape: ctx.enter_context(
            nc.sbuf_tensor(name, shape, mybir.dt.float32)
        )
        ps = lambda name: ctx.enter_context(
            nc.psum_tensor(name, [2, 2048], mybir.dt.float32)
        )
        et = [sb(f"et{i}", [128, FREE]) for i in range(3)]
        mt = [sb(f"mt{i}", [128, FREE]) for i in range(2)]
        dn = sb("dn", [128, W])
        rc = [sb("rc0", [128, W]), sb("rc1", [128, W])]
        sq = [sb(f"sq{i}", [2, 2048]) for i in range(NSQ)]
        wgt = sb("wgt_sb", [128, 2])
        pq = [ps(f"pq{i}") for i in range(2)]

        sem = lambda name: ctx.enter_context(nc.semaphore(name))
        s_wgt = sem("s_wgt")
        s_in = [
            [sem(f"s_in{b}_{h}") for h in range(8 if b == 0 else (4 if b == 3 else 2))]
            for b in range(BLOC)
        ]
        s_exp = sem("s_exp")
        s_dve = sem("s_dve")
        s_pe = sem("s_pe")
        s_sq = sem("s_sq")
        s_out = [sem(f"s_out{b}") for b in range(BLOC)]
        s_pl = sem("s_pl")
        s_cs = [sem(f"s_cs{i}") for i in range(NSQ)]

        block = ctx.enter_context(nc.Block())

        @block.sync
        def _(sp):
            sp.dma_start(wgt[:], wgt_dram).then_inc(s_wgt, 16)

            IN_PARTS = [8, 2, 2, 4]

            def do_in(b):
                nparts = IN_PARTS[b]
                kp = K // nparts
                seg = kp * W
                if b >= 3:
                    # et slot reuse: batch b-3 consumers done
                    sp.wait_ge(s_dve, 3 * (b - 3) + 3)
                    sp.wait_ge(s_pl, b - 2)
                for h in range(nparts):
                    sp.dma_start(
                        et[b % 3][:, h * seg : (h + 1) * seg].rearrange(
                            "p (k w) -> p k w", k=kp
                        ),
                        x[b][h * kp : (h + 1) * kp].transpose([1, 0, 2]),
                    ).then_inc(s_in[b][h], 16)

            def do_out(b):
                sp.wait_ge(s_dve, 3 * b + 3)  # map(b) low-k ready
                if b < 3:
                    sp.wait_ge(s_pl, b + 1)  # map(b) high-k ready
                else:
                    # let batch-2 csums transfers beat out3 into the queue
                    sp.wait_ge(s_cs[8 % NSQ], 32)
                sp.dma_start(
                    map_out[b].transpose([1, 0, 2]),
                    mt[b % 2][:].rearrange("p (k w) -> p k w", k=K),
                ).then_inc(s_out[b], 16)

            do_in(0)
            do_in(1)
            do_in(2)
            do_out(0)
            do_in(3)
            do_out(1)
            do_out(2)
            do_out(3)
            # quiesce
            for i in range(NSQ):
                sp.wait_ge(s_cs[i], 16 * (GQ_TOT // NSQ))

        @block.scalar
        def _(act):
            EXP_PARTS = [8, 2, 2, 4]

            def exp_batch(b):
                nparts = EXP_PARTS[b]
                seglen = FREE // nparts
                for h in range(nparts):
                    act.wait_ge(s_in[b][h], 16)
                    seg = et[b % 3][:, h * seglen : (h + 1) * seglen]
                    act.activation(
                        seg, seg, mybir.ActivationFunctionType.Exp
                    ).then_inc(s_exp, 1)

            def drains(b):
                for q in range(NQ):
                    gq = NQ * b + q
                    act.wait_ge(s_pe, 16 * b + 4 * (q + 1))
                    if gq >= NSQ:
                        act.wait_ge(s_cs[gq % NSQ], 16 * (gq // NSQ))
                    act.activation(
                        sq[gq % NSQ][:],
                        pq[gq % 2][:],
                        mybir.ActivationFunctionType.Copy,
                    ).then_inc(s_sq, 1)

            exp_batch(0)
            exp_batch(1)
            exp_batch(2)
            drains(0)
            exp_batch(3)
            drains(1)
            drains(2)
            drains(3)

        @block.vector
        def _(dve):
            EXP_DONE = [8, 10, 12, 16]
            for b in range(BLOC):
                dve.wait_ge(s_exp, EXP_DONE[b])
                if b >= 1:
                    # dn reuse: batch b-1 recip done
                    dve.wait_ge(s_dve, 3 * (b - 1) + 2)
                dve.tensor_reduce(
                    dn[:],
                    et[b % 3][:].rearrange("p (k w) -> p w k", k=K),
                    axis=mybir.AxisListType.X,
                    op=mybir.AluOpType.add,
                ).then_inc(s_dve, 1)
                dve.wait_ge(s_dve, 3 * b + 1)
                if b >= 2:
                    dve.wait_ge(s_pl, b - 1)  # rc slot: pool mult(b-2) done
                dve.reciprocal(rc[b % 2][:], dn[:]).then_inc(s_dve, 1)
                dve.wait_ge(s_dve, 3 * b + 2)
                if b >= 2:
                    dve.wait_ge(s_out[b - 2], 16)  # mt slot: out(b-2) done
                    dve.wait_ge(s_pe, 16 * (b - 1))  # mt slot: PE(b-2) done
                kd = MKD if b < 3 else K  # batch 3: all of k on DVE
                e3 = et[b % 3][:, : kd * W].rearrange("p (k w) -> p k w", k=kd)
                m3r = (
                    mt[b % 2][:, : kd * W]
                    .rearrange("p (k w) -> p k w", k=kd)
                    .bitcast(mybir.dt.float32r)
                )
                rcb = rc[b % 2][:].unsqueeze(1).broadcast_to([128, kd, W])
                dve.tensor_tensor(m3r, e3, rcb, op=mybir.AluOpType.mult).then_inc(
                    s_dve, 1
                )

        @block.tensor
        def _(pe):
            pe.wait_ge(s_wgt, 16)
            for b in range(BLOC):
                pe.wait_ge(s_dve, 3 * b + 3)
                if b < 3:
                    pe.wait_ge(s_pl, b + 1)
                for q in range(NQ):
                    gq = NQ * b + q
                    if gq >= 2:
                        pe.wait_ge(s_sq, gq - 1)
                    for n in range(4):
                        col = q * 2048 + n * 512
                        pe.matmul(
                            pq[gq % 2][:, n * 512 : (n + 1) * 512],
                            lhsT=wgt[:].bitcast(mybir.dt.float32r),
                            rhs=mt[b % 2][:, col : col + 512].bitcast(
                                mybir.dt.float32r
                            ),
                            start=True,
                            stop=True,
                        ).then_inc(s_pe, 1)

        @block.gpsimd
        def _(pl):
            def pmult(b):
                pl.wait_ge(s_dve, 3 * b + 2)  # recip(b) ready
                if b >= 2:
                    pl.wait_ge(s_out[b - 2], 16)  # mt slot: out(b-2) done
                    pl.wait_ge(s_pe, 16 * (b - 1))  # mt slot: PE(b-2) done
                e3h = et[b % 3][:, MKD * W :].rearrange("p (k w) -> p k w", k=K - MKD)
                m3hr = (
                    mt[b % 2][:, MKD * W :]
                    .rearrange("p (k w) -> p k w", k=K - MKD)
                    .bitcast(mybir.dt.float32r)
                )
                rcbh = rc[b % 2][:].unsqueeze(1).broadcast_to([128, K - MKD, W])
                pl.tensor_tensor(m3hr, e3h, rcbh, op=mybir.AluOpType.mult).then_inc(
                    s_pl, 1
                )

            def pcs1(b, q):
                gq = NQ * b + q
                pl.wait_ge(s_sq, gq + 1)
                if gq >= NSQ:
                    pl.wait_ge(s_cs[gq % NSQ], 16 * (gq // NSQ))
                pl.dma_start(
                    csums[b][:, q * 16 : (q + 1) * 16, :],
                    sq[gq % NSQ][:].rearrange("c (k w) -> c k w", k=16),
                ).then_inc(s_cs[gq % NSQ], 16)

            def pcs(b):
                for q in range(NQ):
                    pcs1(b, q)

            pmult(0)
            pmult(1)
            pcs(0)
            pmult(2)
            pcs(1)
            pcs(2)
            pcs(3)

    _cache["nc"] = nc
    return nc


def kernel(combined_hm_preds, cur_batch=None, num_of_kp=None):
    xfull = np.ascontiguousarray(np.asarray(combined_hm_preds, dtype=np.float32))
    assert xfull.shape == (B, K, H, W)

    nc = _build()
    core_ids = list(range(NCORES))
    in_maps = [{"x": xfull[i * BLOC : (i + 1) * BLOC]} for i in range(NCORES)]

    trace = os.environ.get("KERNEL_TRACE", "0") == "1"
    if trace:
        import gauge.profiler

        with gauge.profiler.profile(
            kernel_dev_mode=True, profile_on_exit=False, bass_kernel=nc.m
        ) as profile:
            res = run_bass_kernel_spmd(nc, in_maps, core_ids, trace=False)
        try:
            pr = profile.to_perfetto(model_index=(0,))
            if pr:
                res.exec_time_ns = pr[0].exec_time_ns
                res.instructions_and_trace = (pr[0].insts, pr[0].trace_path)
            _cache["last_profile"] = profile
        except Exception as e:
            print(f"profiling failed: {e!r}")
    else:
        res = run_bass_kernel_spmd(nc, in_maps, core_ids, trace=False)
    _cache["last_exec_time_ns"] = res.exec_time_ns
    _cache["last_result"] = res

    map_val = np.concatenate(
        [res.results[i]["map_out"] for i in range(NCORES)], axis=0
    )
    csums = np.concatenate(
        [res.results[i]["csums"] for i in range(NCORES)], axis=0
    )  # [B, 2, K, W]

    cs = csums.astype(np.float64)
    get_zeta = cs[:, 0].sum(axis=2).astype(np.float32)  # [B, K]
    kp_y = cs[:, 1].sum(axis=2).astype(np.float32)
    ws = np.arange(W, dtype=np.float64)
    kp_x = (cs[:, 0] * ws).sum(axis=2).astype(np.float32)

    keypoint = np.stack(
        [np.rint(kp_x / get_zeta), np.rint(kp_y / get_zeta)], axis=-1
    ).astype(np.float32)

    return map_val, keypoint, get_zeta


def bench(xfull=None, iters=32, warmup=4):
    """Time back-to-back NEFF executions on the 8 cores; returns ns/iter.

    Inputs stay device-resident; output buffers are donation-chained so
    successive executions serialize on-device without host transfers.
    """
    import time

    import jax
    from jax.experimental.shard_map import shard_map
    from jax.sharding import Mesh, NamedSharding, PartitionSpec

    from concourse import bass2jax as b2j

    if xfull is None:
        xfull = np.zeros((B, K, H, W), np.float32)
    xfull = np.ascontiguousarray(np.asarray(xfull, dtype=np.float32))

    nc = _build()
    b2j.install_neuronx_cc_hook()

    in_names = ["x"]
    out_names = ["map_out", "csums"]
    out_avals = [
        jax.core.ShapedArray((BLOC, K, H, W), np.float32),
        jax.core.ShapedArray((BLOC, 2, K, W), np.float32),
    ]
    all_in_names = in_names + out_names
    partition_name = nc.partition_id_tensor.name if nc.partition_id_tensor else None
    if partition_name is not None:
        all_in_names.append(partition_name)

    def _body(*args):
        operands = list(args)
        if partition_name is not None:
            operands.append(b2j.partition_id_tensor())
        return tuple(
            b2j._bass_exec_p.bind(
                *operands,
                out_avals=tuple(out_avals),
                in_names=tuple(all_in_names),
                out_names=tuple(out_names),
                lowering_input_output_aliases=(),
                sim_require_finite=True,
                sim_require_nnan=True,
                nc=nc,
            )
        )

    devices = jax.devices()[:NCORES]
    mesh = Mesh(np.asarray(devices), ("core",))
    spec = PartitionSpec("core")
    sharded = jax.jit(
        shard_map(
            _body,
            mesh=mesh,
            in_specs=(spec,) * 3,
            out_specs=(spec,) * 2,
            check_rep=False,
        ),
        donate_argnums=(1, 2),
        keep_unused=True,
    )

    shd = NamedSharding(mesh, spec)
    x_dev = jax.device_put(xfull, shd)
    o1 = jax.device_put(np.zeros((B, K, H, W), np.float32), shd)
    o2 = jax.device_put(np.zeros((B, 2, K, W), np.float32), shd)

    for _ in range(warmup):
        o1, o2 = sharded(x_dev, o1, o2)
    jax.block_until_ready((o1, o2))

    t0 = time.perf_counter()
    for _ in range(iters):
        o1, o2 = sharded(x_dev, o1, o2)
    jax.block_until_ready((o1, o2))
    t1 = time.perf_counter()
    return (t1 - t0) / iters * 1e9


# revision 15
# speedup vs baseline: 1.0414x; 1.0414x over previous
"""DetectionConfidenceMap2keypoint Trainium2 kernel (8 NeuronCores).

For x = combined_hm_preds [B=32, K=64, H=128, W=128] f32:
  map_val_all = softmax(x, axis=1)                 # over K, per (b,h,w)
  get_zeta[b,k]  = sum_{h,w} map
  kp_x[b,k]      = sum_{h,w} w * map
  kp_y[b,k]      = sum_{h,w} h * map
  keypoint[b,k]  = (round(kp_x/zeta), round(kp_y/zeta))

Sharding: data-parallel over batch, 4 batches per core on 8 cores (no
communication).

Device layout per batch: SBUF tile [h=128 partitions, (k,w)=8192 free].
Raw bass (no Tile) with a hand-scheduled pipeline and standalone
semaphore waits (this container's walrus encodes at most one inline
wait per instruction, which Tile's scheduler exceeds):
  SP  : dma-in  x[b] -> xt[b%2]
  ACT : exp xt->et; psum drain copies; map dma-out (ACT HWDGE ring)
  DVE : denom[h,w] = reduce_k(exp); recip; map = exp*recip (k-bcast)
  PE  : [ones | h]^T @ map -> colsum/ycolsum per (k,w)  (h-reduction)
  PL  : csums dma-out
Host: w-sums of colsums -> zeta/kp_x/kp_y; divide + round -> keypoint.
"""

import os
from contextlib import ExitStack

import numpy as np

import concourse.bass as bass
import concourse.mybir as mybir
from concourse.bass_utils import run_bass_kernel_spmd

B, K, H, W = 32, 64, 128, 128
MKD = 36  # k's normalized on DVE; rest on Pool (Pool ~2x slower/elem)
NCORES = 8
BLOC = B // NCORES
FREE = K * W  # 8192
HALF = FREE // 2  # 4096
KH = K // 2  # 32 k's per input half
NQ = 4
GQ_TOT = BLOC * NQ
NSQ = 6

_cache = {}



def _build():
    if "nc" in _cache:
        return _cache["nc"]

    nc = bass.Bass(
        target_bir_lowering=False,
        debug=False,
        num_devices=NCORES,
        dynamic_dma_scratch_size=8192,
    )

    x = nc.dram_tensor(
        "x", [BLOC, K, H, W], mybir.dt.float32, kind="ExternalInput"
    ).ap()
    map_out = nc.dram_tensor(
        "map_out", [BLOC, K, H, W], mybir.dt.float32, kind="ExternalOutput"
    ).ap()
    csums = nc.dram_tensor(
        "csums", [BLOC, 2, K, W], mybir.dt.float32, kind="ExternalOutput"
    ).ap()

    wgt_np = np.zeros((128, 2), dtype=np.float32)
    wgt_np[:, 0] = 1.0
    wgt_np[:, 1] = np.arange(128, dtype=np.float32)
    wgt_dram = nc.inline_tensor(wgt_np, "wgt").ap()

    ctx = ExitStack()
    with ctx:
        sb = lambda name, shape: ctx.enter_context(
            nc.sbuf_tensor(name, shape, mybir.dt.float32)
        )
        ps = lambda name: ctx.enter_context(
            nc.psum_tensor(name, [2, 2048], mybir.dt.float32)
        )
        et = [sb(f"et{i}", [128, FREE]) for i in range(3)]
        mt = [sb(f"mt{i}", [128, FREE]) for i in range(2)]
        dn = sb("dn", [128, W])
        rc = [sb("rc0", [128, W]), sb("rc1", [128, W])]
        sq = [sb(f"sq{i}", [2, 2048]) for i in range(NSQ)]
        wgt = sb("wgt_sb", [128, 2])
        pq = [ps(f"pq{i}") for i in range(2)]

        sem = lambda name: ctx.enter_context(nc.semaphore(name))
        s_wgt = sem("s_wgt")
        s_in = [
            [sem(f"s_in{b}_{h}") for h in range(8 if b == 0 else (4 if b == 3 else 2))]
            for b in range(BLOC)
        ]
        s_exp = sem("s_exp")
        s_dve = sem("s_dve")
        s_pe = sem("s_pe")
        s_sq = sem("s_sq")
        s_out = [sem(f"s_out{b}") for b in range(BLOC)]
        s_pl = sem("s_pl")
        s_cs = [sem(f"s_cs{i}") for i in range(NSQ)]
        s_csd = sem("s_csd")  # csums descriptor batches enqueued (counter)
        s_o3b = sem("s_o3b")  # out3 second-half completion

        block = ctx.enter_context(nc.Block())

        @block.sync
        def _(sp):
            IN_PARTS = [8, 2, 2, 4]

            def do_in(b):
                nparts = IN_PARTS[b]
                kp = K // nparts
                seg = kp * W
                if b >= 3:
                    # et slot reuse: batch b-3 consumers done
                    sp.wait_ge(s_dve, 3 * (b - 3) + 3)
                    sp.wait_ge(s_pl, b - 2)
                for h in range(nparts):
                    sp.dma_start(
                        et[b % 3][:, h * seg : (h + 1) * seg].rearrange(
                            "p (k w) -> p k w", k=kp
                        ),
                        x[b][h * kp : (h + 1) * kp].transpose([1, 0, 2]),
                    ).then_inc(s_in[b][h], 16)

            def do_out(b):
                sp.wait_ge(s_dve, 3 * b + 3)  # map(b) low-k ready
                if b < 3:
                    sp.wait_ge(s_pl, b + 1)  # map(b) high-k ready
                if b >= 1:
                    # let csums(b-1) descriptors beat out(b) into the queue
                    sp.wait_ge(s_csd, b)
                if b < 3:
                    sp.dma_start(
                        map_out[b].transpose([1, 0, 2]),
                        mt[b % 2][:].rearrange("p (k w) -> p k w", k=K),
                    ).then_inc(s_out[b], 16)
                else:
                    # first half (k 0..KH) ready at s_dve>=12 (waited above)
                    sp.dma_start(
                        map_out[b][:KH].transpose([1, 0, 2]),
                        mt[b % 2][:, :HALF].rearrange("p (k w) -> p k w", k=KH),
                    ).then_inc(s_out[b], 16)
                    sp.wait_ge(s_dve, 13)  # second half ready
                    sp.dma_start(
                        map_out[b][KH:].transpose([1, 0, 2]),
                        mt[b % 2][:, HALF:].rearrange("p (k w) -> p k w", k=KH),
                    ).then_inc(s_o3b, 16)

            do_in(0)
            sp.dma_start(wgt[:], wgt_dram).then_inc(s_wgt, 16)
            do_in(1)
            do_in(2)
            do_out(0)
            do_in(3)
            do_out(1)
            do_out(2)
            do_out(3)
            # quiesce
            sp.wait_ge(s_o3b, 16)
            for i in range(NSQ):
                sp.wait_ge(s_cs[i], 16 * (GQ_TOT // NSQ))

        @block.scalar
        def _(act):
            EXP_PARTS = [8, 2, 2, 4]

            def exp_batch(b):
                nparts = EXP_PARTS[b]
                seglen = FREE // nparts
                for h in range(nparts):
                    act.wait_ge(s_in[b][h], 16)
                    seg = et[b % 3][:, h * seglen : (h + 1) * seglen]
                    act.activation(
                        seg, seg, mybir.ActivationFunctionType.Exp
                    ).then_inc(s_exp, 1)

            def drain1(b, q):
                gq = NQ * b + q
                act.wait_ge(s_pe, 16 * b + 4 * (q + 1))
                if gq >= NSQ:
                    act.wait_ge(s_cs[gq % NSQ], 16 * (gq // NSQ))
                act.activation(
                    sq[gq % NSQ][:],
                    pq[gq % 2][:],
                    mybir.ActivationFunctionType.Copy,
                ).then_inc(s_sq, 1)

            def drains(b):
                for q in range(NQ):
                    drain1(b, q)

            def exp1(b, h):
                seglen = FREE // EXP_PARTS[b]
                act.wait_ge(s_in[b][h], 16)
                seg = et[b % 3][:, h * seglen : (h + 1) * seglen]
                act.activation(seg, seg, mybir.ActivationFunctionType.Exp).then_inc(
                    s_exp, 1
                )

            exp_batch(0)
            exp_batch(1)
            exp_batch(2)
            drains(0)
            for q in range(4):
                exp1(3, q)  # quarter arrives every ~2.9us; drain fits the gap
                drain1(1, q)
            drains(2)
            drains(3)

        @block.vector
        def _(dve):
            EXP_DONE = [8, 10, 12, 16]
            for b in range(BLOC):
                dve.wait_ge(s_exp, EXP_DONE[b])
                if b >= 1:
                    # dn reuse: batch b-1 recip done
                    dve.wait_ge(s_dve, 3 * (b - 1) + 2)
                dve.tensor_reduce(
                    dn[:],
                    et[b % 3][:].rearrange("p (k w) -> p w k", k=K),
                    axis=mybir.AxisListType.X,
                    op=mybir.AluOpType.add,
                ).then_inc(s_dve, 1)
                dve.wait_ge(s_dve, 3 * b + 1)
                if b >= 2:
                    dve.wait_ge(s_pl, b - 1)  # rc slot: pool mult(b-2) done
                dve.reciprocal(rc[b % 2][:], dn[:]).then_inc(s_dve, 1)
                dve.wait_ge(s_dve, 3 * b + 2)
                if b >= 2:
                    dve.wait_ge(s_out[b - 2], 16)  # mt slot: out(b-2) done
                    dve.wait_ge(s_pe, 16 * (b - 1))  # mt slot: PE(b-2) done
                if b < 3:
                    kd = MKD
                    e3 = et[b % 3][:, : kd * W].rearrange("p (k w) -> p k w", k=kd)
                    m3r = (
                        mt[b % 2][:, : kd * W]
                        .rearrange("p (k w) -> p k w", k=kd)
                        .bitcast(mybir.dt.float32r)
                    )
                    rcb = rc[b % 2][:].unsqueeze(1).broadcast_to([128, kd, W])
                    dve.tensor_tensor(
                        m3r, e3, rcb, op=mybir.AluOpType.mult
                    ).then_inc(s_dve, 1)
                else:
                    # batch 3: all of k on DVE, two half-K chunks so the
                    # map-out can start on the first half
                    for c in range(2):
                        lo, hi = c * HALF, (c + 1) * HALF
                        e3 = et[b % 3][:, lo:hi].rearrange(
                            "p (k w) -> p k w", k=KH
                        )
                        m3r = (
                            mt[b % 2][:, lo:hi]
                            .rearrange("p (k w) -> p k w", k=KH)
                            .bitcast(mybir.dt.float32r)
                        )
                        rcb = rc[b % 2][:].unsqueeze(1).broadcast_to(
                            [128, KH, W]
                        )
                        if c == 1:
                            dve.wait_ge(s_dve, 12)
                        dve.tensor_tensor(
                            m3r, e3, rcb, op=mybir.AluOpType.mult
                        ).then_inc(s_dve, 1)

        @block.tensor
        def _(pe):
            pe.wait_ge(s_wgt, 16)
            for b in range(BLOC):
                pe.wait_ge(s_dve, 3 * b + 3 if b < 3 else 12)
                if b < 3:
                    pe.wait_ge(s_pl, b + 1)
                for q in range(NQ):
                    gq = NQ * b + q
                    if b == 3 and q == 2:
                        pe.wait_ge(s_dve, 13)  # second mult(3) chunk
                    if gq >= 2:
                        pe.wait_ge(s_sq, gq - 1)
                    for n in range(4):
                        col = q * 2048 + n * 512
                        pe.matmul(
                            pq[gq % 2][:, n * 512 : (n + 1) * 512],
                            lhsT=wgt[:].bitcast(mybir.dt.float32r),
                            rhs=mt[b % 2][:, col : col + 512].bitcast(
                                mybir.dt.float32r
                            ),
                            start=True,
                            stop=True,
                        ).then_inc(s_pe, 1)

        @block.gpsimd
        def _(pl):
            def pmult(b):
                pl.wait_ge(s_dve, 3 * b + 2)  # recip(b) ready
                if b >= 2:
                    pl.wait_ge(s_out[b - 2], 16)  # mt slot: out(b-2) done
                    pl.wait_ge(s_pe, 16 * (b - 1))  # mt slot: PE(b-2) done
                e3h = et[b % 3][:, MKD * W :].rearrange("p (k w) -> p k w", k=K - MKD)
                m3hr = (
                    mt[b % 2][:, MKD * W :]
                    .rearrange("p (k w) -> p k w", k=K - MKD)
                    .bitcast(mybir.dt.float32r)
                )
                rcbh = rc[b % 2][:].unsqueeze(1).broadcast_to([128, K - MKD, W])
                pl.tensor_tensor(m3hr, e3h, rcbh, op=mybir.AluOpType.mult).then_inc(
                    s_pl, 1
                )

            def pcs1(b, q):
                gq = NQ * b + q
                pl.wait_ge(s_sq, gq + 1)
                if gq >= NSQ:
                    pl.wait_ge(s_cs[gq % NSQ], 16 * (gq // NSQ))
                pl.dma_start(
                    csums[b][:, q * 16 : (q + 1) * 16, :],
                    sq[gq % NSQ][:].rearrange("c (k w) -> c k w", k=16),
                ).then_inc(s_cs[gq % NSQ], 16)

            def pcs(b):
                for q in range(NQ):
                    pcs1(b, q)

            pmult(0)
            pmult(1)
            pcs(0)
            pl.sem_inc(s_csd, 1)
            pmult(2)
            pcs(1)
            pl.sem_inc(s_csd, 1)
            pcs1(2, 0)
            pcs1(2, 1)
            pl.sem_inc(s_csd, 1)
            pcs1(2, 2)
            pcs1(2, 3)
            pcs(3)

    _cache["nc"] = nc
    return nc


def kernel(combined_hm_preds, cur_batch=None, num_of_kp=None):
    xfull = np.ascontiguousarray(np.asarray(combined_hm_preds, dtype=np.float32))
    assert xfull.shape == (B, K, H, W)

    nc = _build()
    core_ids = list(range(NCORES))
    in_maps = [{"x": xfull[i * BLOC : (i + 1) * BLOC]} for i in range(NCORES)]

    trace = os.environ.get("KERNEL_TRACE", "0") == "1"
    if trace:
        import gauge.profiler

        with gauge.profiler.profile(
            kernel_dev_mode=True, profile_on_exit=False, bass_kernel=nc.m
        ) as profile:
            res = run_bass_kernel_spmd(nc, in_maps, core_ids, trace=False)
        try:
            pr = profile.to_perfetto(model_index=(0,))
            if pr:
                res.exec_time_ns = pr[0].exec_time_ns
                res.instructions_and_trace = (pr[0].insts, pr[0].trace_path)
            _cache["last_profile"] = profile
        except Exception as e:
            print(f"profiling failed: {e!r}")
    else:
        res = run_bass_kernel_spmd(nc, in_maps, core_ids, trace=False)
    _cache["last_exec_time_ns"] = res.exec_time_ns
    _cache["last_result"] = res

    map_val = np.concatenate(
        [res.results[i]["map_out"] for i in range(NCORES)], axis=0
    )
    csums = np.concatenate(
        [res.results[i]["csums"] for i in range(NCORES)], axis=0
    )  # [B, 2, K, W]

    cs = csums.astype(np.float64)
    get_zeta = cs[:, 0].sum(axis=2).astype(np.float32)  # [B, K]
    kp_y = cs[:, 1].sum(axis=2).astype(np.float32)
    ws = np.arange(W, dtype=np.float64)
    kp_x = (cs[:, 0] * ws).sum(axis=2).astype(np.float32)

    keypoint = np.stack(
        [np.rint(kp_x / get_zeta), np.rint(kp_y / get_zeta)], axis=-1
    ).astype(np.float32)

    return map_val, keypoint, get_zeta


def bench(xfull=None, iters=32, warmup=4):
    """Time back-to-back NEFF executions on the 8 cores; returns ns/iter.

    Inputs stay device-resident; output buffers are donation-chained so
    successive executions serialize on-device without host transfers.
    """
    import time

    import jax
    from jax.experimental.shard_map import shard_map
    from jax.sharding import Mesh, NamedSharding, PartitionSpec

    from concourse import bass2jax as b2j

    if xfull is None:
        xfull = np.zeros((B, K, H, W), np.float32)
    xfull = np.ascontiguousarray(np.asarray(xfull, dtype=np.float32))

    nc = _build()
    b2j.install_neuronx_cc_hook()

    in_names = ["x"]
    out_names = ["map_out", "csums"]
    out_avals = [
        jax.core.ShapedArray((BLOC, K, H, W), np.float32),
        jax.core.ShapedArray((BLOC, 2, K, W), np.float32),
    ]
    all_in_names = in_names + out_names
    partition_name = nc.partition_id_tensor.name if nc.partition_id_tensor else None
    if partition_name is not None:
        all_in_names.append(partition_name)

    def _body(*args):
        operands = list(args)
        if partition_name is not None:
            operands.append(b2j.partition_id_tensor())
        return tuple(
            b2j._bass_exec_p.bind(
                *operands,
                out_avals=tuple(out_avals),
                in_names=tuple(all_in_names),
                out_names=tuple(out_names),
                lowering_input_output_aliases=(),
                sim_require_finite=True,
                sim_require_nnan=True,
                nc=nc,
            )
        )

    devices = jax.devices()[:NCORES]
    mesh = Mesh(np.asarray(devices), ("core",))
    spec = PartitionSpec("core")
    sharded = jax.jit(
        shard_map(
            _body,
            mesh=mesh,
            in_specs=(spec,) * 3,
            out_specs=(spec,) * 2,
            check_rep=False,
        ),
        donate_argnums=(1, 2),
        keep_unused=True,
    )

    shd = NamedSharding(mesh, spec)
    x_dev = jax.device_put(xfull, shd)
    o1 = jax.device_put(np.zeros((B, K, H, W), np.float32), shd)
    o2 = jax.device_put(np.zeros((B, 2, K, W), np.float32), shd)

    for _ in range(warmup):
        o1, o2 = sharded(x_dev, o1, o2)
    jax.block_until_ready((o1, o2))

    t0 = time.perf_counter()
    for _ in range(iters):
        o1, o2 = sharded(x_dev, o1, o2)
    jax.block_until_ready((o1, o2))
    t1 = time.perf_counter()
    return (t1 - t0) / iters * 1e9
